# revision 12
# baseline (speedup 1.0000x reference)
"""Trainium2 Bass kernel for EvollaSequenceCompressorAttention.

Computation (per batch element):
    xn = LN(x, gm, bm); ln = LN(latents, gl, bl)
    q  = (ln @ Wq) * dh**-0.5
    kv = concat(xn, ln) @ Wkv ; k, v = split(kv)
    sim = q_h @ k_h^T  (per head), masked softmax over keys
    out = (attn @ v_h heads merged) @ Wo

Strategy:
  - Data-parallel over batch: 16 batch elements -> 8 cores x 2.
  - LayerNorm folded into the matmuls:
        xn @ (diag(g) Wkv) = (x*rstd) @ Wg + alpha (x) (g@Wkv) + 1 (x) (b@Wkv)
    with alpha = -mu*rstd.  Host pre-scales weights by g (and the q-scale);
    the rank-2 correction rides the matmul as a K=2 augmented tail using an
    on-chip [alpha; ones] row pair.
  - x*rstd is computed token-major (per-partition scale on ACT), cast to fp16,
    and transposed to feature-major via SBUF->SBUF DMA xbar transpose.
  - k produced feature-major (k^T), v token-major, so attention needs no
    operand transposes; attn (post softmax) is DMA-transposed for the AV
    matmul.  Mask enters the sim PSUM as a K=1 matmul bias of -30000.
  - All matmuls fp16 inputs, fp32 PSUM accumulation.
"""

import sys

sys.path.insert(0, "/opt/trn_rl_repo")

import numpy as np

DIM = 1024
DH = 64
HEADS = 8
INNER = DH * HEADS  # 512
N1 = 2048
N2 = 64
NK = N1 + N2  # 2112
NKP = 2176  # NK padded to multiple of 128
B = 16
NCORES = 8
BPC = B // NCORES  # batches per core
EPS = 1e-5
SCALE = DH**-0.5
P = 128
MASK_NEG = -30000.0
F16 = "float16"


def build_module():
    """Build the Bass module for ONE core processing BPC batch elements."""
    import concourse.bass as bass
    import concourse.mybir as mybir
    import concourse.tile as tile
    from concourse import bacc

    f32 = mybir.dt.float32
    f16 = mybir.dt.float16
    i32 = mybir.dt.int32
    ADD = mybir.AluOpType.add
    MULT = mybir.AluOpType.mult
    MAX = mybir.AluOpType.max
    MIN = mybir.AluOpType.min
    AX = mybir.AxisListType.X
    ACTF = mybir.ActivationFunctionType

    nc = bacc.Bacc()

    x_d = nc.declare_dram_parameter("x", [BPC, N1, DIM], f32, isOutput=False)
    lat_d = nc.declare_dram_parameter("lat", [BPC, N2, DIM], f32, isOutput=False)
    mask_d = nc.declare_dram_parameter("mask", [BPC, NK], i32, isOutput=False)
    wkv_d = nc.declare_dram_parameter("wkv", [P, 8, 2 * INNER], f16, isOutput=False)
    wq_d = nc.declare_dram_parameter("wq", [P, 8, INNER], f16, isOutput=False)
    wo_d = nc.declare_dram_parameter("wo", [P, 4, DIM], f16, isOutput=False)
    awkv_d = nc.declare_dram_parameter("awkv", [2, 2 * INNER], f16, isOutput=False)
    awkvl_d = nc.declare_dram_parameter("awkvl", [2, 2 * INNER], f16, isOutput=False)
    awq_d = nc.declare_dram_parameter("awq", [2, INNER], f16, isOutput=False)
    ratio_d = nc.declare_dram_parameter("ratio", [P, 8], f32, isOutput=False)
    out_d = nc.declare_dram_parameter("out", [BPC, N2, DIM], f32, isOutput=True)

    alpha_scr = nc.dram_tensor("alpha_scr", [BPC, NKP], f32)

    NT = N1 // P  # 16 x token tiles
    NTT = NT + 1  # + latents tile

    with tile.TileContext(nc) as tc:
        with (
            tc.tile_pool(name="wpool", bufs=1) as wpool,
            tc.tile_pool(name="big", bufs=1) as big,
            tc.tile_pool(name="stream", bufs=3) as stream,
            tc.tile_pool(name="mid", bufs=2) as mid,
            tc.tile_pool(name="rows1", bufs=1) as rows1,
            tc.tile_pool(name="small", bufs=4) as small,
            tc.tile_pool(name="psmall", bufs=3, space="PSUM") as psmall,
            tc.tile_pool(name="psim", bufs=1, space="PSUM") as psim,
        ):
            # ---- persistent weights ----
            wkv = wpool.tile([P, 8, 2 * INNER], f16, tag="wkv")
            nc.sync.dma_start(wkv, wkv_d[:])
            wq = wpool.tile([P, 8, INNER], f16, tag="wq")
            nc.sync.dma_start(wq, wq_d[:])
            wo = wpool.tile([P, 4, DIM], f16, tag="wo")
            nc.sync.dma_start(wo, wo_d[:])
            awkv = wpool.tile([2, 2 * INNER], f16, tag="awkv")
            nc.sync.dma_start(awkv, awkv_d[:])
            awkvl = wpool.tile([2, 2 * INNER], f16, tag="awkvl")
            nc.sync.dma_start(awkvl, awkvl_d[:])
            awq = wpool.tile([2, INNER], f16, tag="awq")
            nc.sync.dma_start(awq, awq_d[:])
            ratio = wpool.tile([P, 8], f32, tag="ratio")
            nc.sync.dma_start(ratio, ratio_d[:])
            ones_col = wpool.tile([1, P], f16, tag="ones_col")
            nc.vector.memset(ones_col, 1.0)

            for b in range(BPC):
                # ---- mask -> bias row: (m-1)*30000 in fp16 ----
                mask_i = rows1.tile([1, NK], i32, tag="mask_i")
                nc.sync.dma_start(mask_i, mask_d[b].unsqueeze(0))
                mask_f = rows1.tile([1, NK], f32, tag="mask_f")
                nc.vector.tensor_copy(mask_f, mask_i)
                maskb = mid.tile([1, NK], f16, tag="maskb")
                nc.vector.tensor_scalar(mask_f, mask_f, -1.0, -MASK_NEG, ADD, MULT)
                nc.vector.tensor_copy(maskb, mask_f)

                # ---- phase A: layernorm stats + x*rstd (fp16) + transpose ----
                xrT = big.tile([P, 8, NK], f16, tag="xrT")
                alpha_tok = mid.tile([P, NTT], f32, tag="alpha_tok")
                nc.vector.memset(alpha_tok[N2:, NT : NT + 1], 0.0)

                for i in range(NTT):
                    if i < NT:
                        pt = P
                        xt = stream.tile([P, DIM], f32, tag="x_in")
                        nc.sync.dma_start(xt, x_d[b, i * P : (i + 1) * P, :])
                    else:
                        pt = N2
                        xt = stream.tile([P, DIM], f32, tag="x_in")
                        nc.sync.dma_start(xt[:pt], lat_d[b])
                    xs = xt[:pt]
                    # bn_stats over two 512-wide groups -> mean/var
                    bst = small.tile([P, 2, 6], f32, tag="bst")
                    nc.vector.bn_stats(bst[:pt, 0], xs[:, :512])
                    nc.vector.bn_stats(bst[:pt, 1], xs[:, 512:])
                    agg = small.tile([P, 2], f32, tag="agg")
                    nc.vector.bn_aggr(agg[:pt], bst[:pt])
                    # rstd = 1/sqrt(var+eps); alpha = -mean*rstd
                    veps = small.tile([P, 1], f32, tag="veps")
                    nc.vector.tensor_scalar_add(veps[:pt], agg[:pt, 1:2], EPS)
                    nc.scalar.sqrt(veps[:pt], veps[:pt])
                    rstd = small.tile([P, 1], f32, tag="rstd")
                    nc.vector.reciprocal(rstd[:pt], veps[:pt])
                    nc.vector.tensor_scalar(
                        alpha_tok[:pt, i : i + 1], agg[:pt, 0:1], rstd[:pt], -1.0,
                        MULT, MULT,
                    )
                    # xr = x * rstd (fp16) ; transpose into xrT
                    xr = stream.tile([P, DIM], f16, tag="xr")
                    nc.scalar.activation(xr[:pt], xs, ACTF.Copy, scale=rstd[:pt])
                    nc.sync.dma_start_transpose(
                        xrT[:, :, i * P : i * P + pt], xr[:pt]
                    )

                # latent rows were scaled for gm; rescale to gl per feature
                for c in range(8):
                    nc.vector.tensor_scalar_mul(
                        xrT[:, c, N1:], xrT[:, c, N1:], ratio[:, c : c + 1]
                    )

                # ---- alpha round trip -> [alpha; ones] fp16 rows ----
                with nc.allow_non_contiguous_dma(reason="alpha staging transpose"):
                    nc.sync.dma_start(
                        alpha_scr[b].rearrange("(t p) -> p t", p=P), alpha_tok
                    )
                alpha_row = rows1.tile([1, NK], f32, tag="alpha_row")
                nc.sync.dma_start(alpha_row, alpha_scr[b, :NK].unsqueeze(0))
                aug = mid.tile([2, NK], f16, tag="aug")
                nc.vector.memset(aug, 1.0)
                nc.vector.tensor_copy(aug[0:1], alpha_row)

                # ---- kv + q matmuls ----
                NCH = [(0, 512), (512, 512), (1024, 512), (1536, 512), (2048, N2)]
                kT = big.tile([P, 4, NK], f16, tag="kT")
                for m in range(4):
                    for n0, nn in NCH:
                        ps = psmall.tile([P, 512], f32, tag="mm512")
                        for c in range(8):
                            nc.tensor.matmul(
                                ps[:, :nn],
                                wkv[:, c, m * P : (m + 1) * P],
                                xrT[:, c, n0 : n0 + nn],
                                start=(c == 0),
                                stop=False,
                            )
                        aw = awkv if n0 < N1 else awkvl
                        nc.tensor.matmul(
                            ps[:, :nn],
                            aw[:, m * P : (m + 1) * P],
                            aug[:, n0 : n0 + nn],
                            start=False,
                            stop=True,
                        )
                        nc.scalar.copy(kT[:, m, n0 : n0 + nn], ps[:, :nn])

                v_sb = big.tile([P, NTT, INNER], f16, tag="v_sb")
                nc.vector.memset(v_sb[N2:, NT], 0.0)
                for i in range(NTT):
                    pt = P if i < NT else N2
                    ps = psmall.tile([P, 512], f32, tag="mm512")
                    for c in range(8):
                        nc.tensor.matmul(
                            ps[:pt],
                            xrT[:, c, i * P : i * P + pt],
                            wkv[:, c, INNER:],
                            start=(c == 0),
                            stop=False,
                        )
                    aw = awkv if i < NT else awkvl
                    nc.tensor.matmul(
                        ps[:pt],
                        aug[:, i * P : i * P + pt],
                        aw[:, INNER:],
                        start=False,
                        stop=True,
                    )
                    nc.vector.tensor_copy(v_sb[:pt, i], ps[:pt])

                qT = mid.tile([P, 4, N2], f16, tag="qT")
                for m in range(4):
                    ps = psmall.tile([P, 512], f32, tag="mm512")
                    for c in range(8):
                        nc.tensor.matmul(
                            ps[:, :N2],
                            wq[:, c, m * P : (m + 1) * P],
                            xrT[:, c, N1:],
                            start=(c == 0),
                            stop=False,
                        )
                    nc.tensor.matmul(
                        ps[:, :N2],
                        awq[:, m * P : (m + 1) * P],
                        aug[:, N1:],
                        start=False,
                        stop=True,
                    )
                    nc.scalar.copy(qT[:, m], ps[:, :N2])

                # ---- attention, head pairs ----
                outT = mid.tile([P, 4, N2], f16, tag="outT")
                for p in range(4):
                    sim = psim.tile([P, NK], f32, tag="sim")
                    for ic, (n0, nn) in enumerate(NCH):
                        nc.tensor.matmul(
                            sim[0:64, n0 : n0 + nn],
                            qT[0:64, p],
                            kT[0:64, p, n0 : n0 + nn],
                            start=True,
                            stop=True,
                            tile_position=(0, 0),
                        )
                        nc.tensor.matmul(
                            sim[64:, n0 : n0 + nn],
                            qT[64:, p],
                            kT[64:, p, n0 : n0 + nn],
                            start=True,
                            stop=True,
                            tile_position=(64, 64),
                        )
                        nc.tensor.matmul(
                            sim[:, n0 : n0 + nn],
                            ones_col,
                            maskb[:, n0 : n0 + nn],
                            start=False,
                            stop=True,
                            skip_group_check=True,
                        )
                    # negmax over chunks
                    nmx = small.tile([P, 5], f32, tag="nmx")
                    for ic, (n0, nn) in enumerate(NCH):
                        nc.vector.tensor_reduce(
                            nmx[:, ic : ic + 1], sim[:, n0 : n0 + nn], AX, MAX,
                            negate=True,
                        )
                    negm = small.tile([P, 1], f32, tag="negm")
                    nc.vector.tensor_reduce(negm, nmx, AX, MIN)
                    # exp + per-chunk sums
                    attn = mid.tile([P, NKP], f16, tag="attn")
                    zc = small.tile([P, 5], f32, tag="zc")
                    for ic, (n0, nn) in enumerate(NCH):
                        nc.scalar.activation(
                            attn[:, n0 : n0 + nn], sim[:, n0 : n0 + nn], ACTF.Exp,
                            bias=negm, accum_out=zc[:, ic : ic + 1],
                        )
                    z = small.tile([P, 1], f32, tag="z")
                    nc.vector.tensor_reduce(z, zc, AX, ADD)
                    rz = small.tile([P, 1], f32, tag="rz")
                    nc.vector.reciprocal(rz, z)
                    nc.vector.tensor_scalar_mul(attn[:, :NK], attn[:, :NK], rz)
                    nc.vector.memset(attn[:, NK:], 0.0)
                    attnT = mid.tile([P, NKP // P, P], f16, tag="attnT")
                    nc.sync.dma_start_transpose(attnT, attn)

                    # AV per head in the pair
                    for hh in range(2):
                        h = 2 * p + hh
                        pav = psmall.tile([P, 512], f32, tag="mm512")
                        av = pav[:DH, :N2]
                        for c in range(NTT):
                            nc.tensor.matmul(
                                av,
                                v_sb[:, c, h * DH : (h + 1) * DH],
                                attnT[:, c, hh * 64 : (hh + 1) * 64],
                                start=(c == 0),
                                stop=(c == NTT - 1),
                            )
                        nc.scalar.copy(outT[hh * 64 : (hh + 1) * 64, p], av)

                # ---- output projection ----
                final = mid.tile([N2, DIM], f32, tag="final")
                for n in range(2):
                    ps = psmall.tile([P, 512], f32, tag="mm512")
                    po = ps[:N2]
                    for c in range(4):
                        nc.tensor.matmul(
                            po,
                            outT[:, c],
                            wo[:, c, n * 512 : (n + 1) * 512],
                            start=(c == 0),
                            stop=(c == 3),
                        )
                    nc.scalar.copy(final[:, n * 512 : (n + 1) * 512], po)
                nc.sync.dma_start(out_d[b], final)

    nc.finalize()
    return nc


def host_prep(inputs):
    """Host-side weight folding + per-core input maps."""
    x = np.asarray(inputs["x"], np.float32)
    latents = np.asarray(inputs["latents"], np.float32)
    mask = np.asarray(inputs["mask"], np.int32)
    gm = np.asarray(inputs["gm"], np.float32)
    bm = np.asarray(inputs["bm"], np.float32)
    gl = np.asarray(inputs["gl"], np.float32)
    bl = np.asarray(inputs["bl"], np.float32)
    Wq = np.asarray(inputs["Wq"], np.float32)
    Wkv = np.asarray(inputs["Wkv"], np.float32)
    Wo = np.asarray(inputs["Wo"], np.float32)

    # x rows use (gm, bm); latent rows use (gl, bl).  The kv matmul streams
    # every token through diag(gm)·Wkv; latent tokens are rescaled on-chip by
    # ratio = gl/gm per feature (exact whenever gm != 0) and use their own
    # augmented rows.
    wkv_g = gm[:, None] * Wkv
    wgsum_kv_x = gm @ Wkv
    wb_kv_x = bm @ Wkv
    wgsum_kv_l = gl @ Wkv
    wb_kv_l = bl @ Wkv
    ratio = np.where(gm != 0.0, gl / np.where(gm != 0.0, gm, 1.0), 0.0)

    wq_g = (gl[:, None] * Wq) * SCALE
    wgsum_q = (gl @ Wq) * SCALE
    wb_q = (bl @ Wq) * SCALE

    def to_pcn(w, chunks):
        # [DIM or INNER rows, n] -> [P, chunks, n] with row = c*P + p
        n = w.shape[1]
        return np.ascontiguousarray(
            w.reshape(chunks, P, n).transpose(1, 0, 2)
        )

    wkv_sb = to_pcn(wkv_g, 8).astype(np.float16)
    wq_sb = to_pcn(wq_g, 8).astype(np.float16)
    wo_sb = to_pcn(Wo, 4).astype(np.float16)
    awkv = np.stack([wgsum_kv_x, wb_kv_x]).astype(np.float16)
    awkvl = np.stack([wgsum_kv_l, wb_kv_l]).astype(np.float16)
    awq = np.stack([wgsum_q, wb_q]).astype(np.float16)
    ratio_sb = np.ascontiguousarray(
        ratio.reshape(8, P).T.astype(np.float32)
    )  # [P, 8], feature d = c*P + p

    in_maps = []
    for core in range(NCORES):
        sl = slice(core * BPC, (core + 1) * BPC)
        in_maps.append(
            {
                "x": np.ascontiguousarray(x[sl]),
                "lat": np.ascontiguousarray(latents[sl]),
                "mask": np.ascontiguousarray(mask[sl]),
                "wkv": wkv_sb,
                "wq": wq_sb,
                "wo": wo_sb,
                "awkv": awkv,
                "awkvl": awkvl,
                "awq": awq,
                "ratio": ratio_sb,
            }
        )
    return in_maps


_NC_CACHE = None


def kernel(**inputs):
    global _NC_CACHE
    from concourse.bass_utils import run_bass_kernel_spmd

    if _NC_CACHE is None:
        _NC_CACHE = build_module()
    nc = _NC_CACHE
    in_maps = host_prep(inputs)
    res = run_bass_kernel_spmd(nc, in_maps, list(range(NCORES)))
    outs = [res.results[i]["out"] for i in range(NCORES)]
    return np.concatenate(outs, axis=0).astype(np.float32)
